# revision 8
# baseline (speedup 1.0000x reference)
"""Trainium2 Bass kernel for AdvancedLSTMCell (B=16384, IN=512, H=1024).

Data-parallel over batch across 8 NeuronCores (2048 rows each). All compute in
a transposed layout (features on partitions, batch on the free dim):

  gatesT[4H, b] = W_gates^T.T @ combinedT    (K = IN+H = 1536)
  c = sig(f)*c_prev + sig(i)*tanh(g); h_pre = sig(o)*tanh(c)
  logits[1, b] = W_a @ h_pre -> exp -> local sum -> AllReduce(add) -> 1/S
  highwayT = [W_ht|W_hg]^T @ xT; h = s*t + (1-s)*h_pre*exp(l)*(1/S)

v3: matmul streams cycle all 8 PSUM banks (4 gates x 2 batch chunks in
flight) -- measured ~25% faster per MM than 4-bank cycling.  i/o gates can run
as fp8e4m3 DoubleRow (K=256 per MM, half the instructions; weights pre-scaled
x64 against fp8 subnormals, undone by the eviction's activation scale).  All
HBM streams are bf16; elementwise work is split DVE/GPSIMD in wide [128,2048]
ops; the 1/S collective overlaps the highway GEMMs.
"""

import numpy as np
import ml_dtypes

import concourse.bass as bass
from concourse import bacc
import concourse.mybir as mybir
import concourse.tile as tile
from concourse.bass_utils import run_bass_kernel_spmd

F32 = mybir.dt.float32
BF16 = mybir.dt.bfloat16
FP8 = mybir.dt.float8e4
AF = mybir.ActivationFunctionType
ALU = mybir.AluOpType
DR = mybir.MatmulPerfMode.DoubleRow

B, IN, H = 16384, 512, 1024
NCORES = 8
BL = B // NCORES          # 2048 rows per core
NB = 4                    # batch chunks of 512 columns
BC = BL // NB             # 512
KG = (IN + H) // 128      # 12 k-tiles for the gates GEMM
KH = IN // 128            # 4 k-tiles for the highway GEMM
NJ = H // 128             # 8 h-blocks
WS = 64.0                 # fp8 weight pre-scale

_cached = {}


def build_program(reps: int = 1, single: bool = False, nocc: bool = False,
                  fp8io: bool = True, mmonly: bool = False,
                  noact: bool = False):
    nc = bacc.Bacc("TRN2", target_bir_lowering=False, debug=False,
                   num_devices=1 if single else NCORES)

    xt = nc.dram_tensor("xt", [KH, 128, BL], BF16, kind="ExternalInput").ap()
    ht = nc.dram_tensor("ht", [NJ, 128, BL], BF16, kind="ExternalInput").ap()
    ct = nc.dram_tensor("ct", [NJ, 128, BL], BF16, kind="ExternalInput").ap()
    wg = nc.dram_tensor("wg", [NJ, KG, 128, 512], BF16, kind="ExternalInput").ap()
    wh = nc.dram_tensor("wh", [NJ, KH, 128, 256], BF16, kind="ExternalInput").ap()
    wa = nc.dram_tensor("wa", [128, NJ], BF16, kind="ExternalInput").ap()
    bg = nc.dram_tensor("bg", [128, 4 * NJ], F32, kind="ExternalInput").ap()
    bh = nc.dram_tensor("bh", [128, 2 * NJ], F32, kind="ExternalInput").ap()
    ba = nc.dram_tensor("ba", [1, 1], F32, kind="ExternalInput").ap()
    x8 = nc.dram_tensor("x8", [KH, 128, BL], FP8, kind="ExternalInput").ap()
    h8 = nc.dram_tensor("h8", [NJ, 128, BL], FP8, kind="ExternalInput").ap()
    w8 = nc.dram_tensor("w8", [NJ, KG // 2, 128, 2, 256], FP8,
                        kind="ExternalInput").ap()
    ho = nc.dram_tensor("ho", [NJ, 128, BL], BF16, kind="ExternalOutput").ap()
    co = nc.dram_tensor("co", [NJ, 128, BL], BF16, kind="ExternalOutput").ap()

    with tile.TileContext(nc) as tc:
        with (
            tc.tile_pool(name="const", bufs=1) as const,
            tc.tile_pool(name="wgp", bufs=2) as wgp,
            tc.tile_pool(name="whp", bufs=2) as whp,
            tc.tile_pool(name="cpp", bufs=2) as cpp,
            tc.tile_pool(name="gact", bufs=2) as gact,
            tc.tile_pool(name="ttp", bufs=7) as ttp,
            tc.tile_pool(name="stp", bufs=2) as stp,
            tc.tile_pool(name="small", bufs=1) as small,
            tc.tile_pool(name="ps", bufs=8, space="PSUM") as psp,
            tc.tile_pool(name="dram", bufs=2, space="DRAM") as dramp,
        ):
            # ---- resident tensors ----
            xt_sb = const.tile([128, KH, BL], BF16, tag="xt_sb")
            ht_sb = const.tile([128, NJ, BL], BF16, tag="ht_sb")
            hpre = const.tile([128, NJ, BL], BF16, tag="hpre")
            expl_bc = const.tile([128, BL], BF16, tag="expl_bc")
            wa_sb = const.tile([128, NJ], BF16, tag="wa_sb")
            bg_sb = const.tile([128, 4 * NJ], F32, tag="bg_sb")
            bh_sb = const.tile([128, 2 * NJ], F32, tag="bh_sb")
            ba_sb = const.tile([1, 1], F32, tag="ba_sb")

            for k in range(KH):
                nc.scalar.dma_start(xt_sb[:, k, :], xt[k])
            for j in range(NJ):
                nc.scalar.dma_start(ht_sb[:, j, :], ht[j])
            nc.scalar.dma_start(wa_sb, wa)
            nc.scalar.dma_start(bg_sb, bg)
            nc.scalar.dma_start(bh_sb, bh)
            nc.scalar.dma_start(ba_sb, ba)
            if fp8io:
                x8_sb = const.tile([128, KH, BL], FP8, tag="x8_sb")
                h8_sb = const.tile([128, NJ, BL], FP8, tag="h8_sb")
                for k in range(KH):
                    nc.scalar.dma_start(x8_sb[:, k, :], x8[k])
                for j in range(NJ):
                    nc.scalar.dma_start(h8_sb[:, j, :], h8[j])

            for _ in range(reps):
                # ================= Phase A: gates + cell update =============
                for j in range(NJ):
                    wgs = wgp.tile([128, KG, 512], BF16, tag="wgs")
                    for q in range(4):
                        nc.sync.dma_start(
                            wgs[:, 3 * q:3 * q + 3, :],
                            wg[j, 3 * q:3 * q + 3].rearrange("k p n -> p k n"))
                    if fp8io:
                        w8s = wgp.tile([128, KG // 2, 2, 256], FP8, tag="w8s")
                        nc.sync.dma_start(
                            w8s, w8[j].rearrange("s p two n -> p s two n"))
                    cp = cpp.tile([128, BL], BF16, tag="cp")
                    nc.sync.dma_start(cp[:, :BL // 2], ct[j][:, :BL // 2])
                    nc.sync.dma_start(cp[:, BL // 2:], ct[j][:, BL // 2:])
                    gfunc = {0: AF.Sigmoid, 1: AF.Sigmoid, 2: AF.Sigmoid,
                             3: AF.Tanh}
                    halves = []
                    for bp in range(2):
                        slh = slice(bp * BL // 2, (bp + 1) * BL // 2)
                        it_ = gact.tile([128, BL // 2], BF16, tag="it")
                        ft_ = gact.tile([128, BL // 2], BF16, tag="ft")
                        ot_ = gact.tile([128, BL // 2], BF16, tag="ot")
                        gt_ = gact.tile([128, BL // 2], BF16, tag="gt")
                        gtiles = {0: it_, 1: ft_, 2: ot_, 3: gt_}
                        halves.append((slh, it_, ft_, ot_, gt_))
                        pg = {}
                        for b2 in range(2):
                            for g in range(4):
                                pg[(b2, g)] = psp.tile(
                                    [128, BC], F32, tag="ps",
                                    name=f"pg{j}_{bp}_{b2}_{g}")
                        if fp8io:
                            # i,o as fp8 DoubleRow (K=256/step); f,g bf16
                            for s in range(KG // 2):
                                for b2 in range(2):
                                    b4 = bp * 2 + b2
                                    sl = slice(b4 * BC, (b4 + 1) * BC)
                                    k0, k1 = 2 * s, 2 * s + 1
                                    if k1 < KH:
                                        rhs8 = x8_sb[:, k0:k1 + 1, sl]
                                    else:
                                        rhs8 = h8_sb[:, k0 - KH:k1 - KH + 1, sl]
                                    r0 = (xt_sb[:, k0, sl] if k0 < KH
                                          else ht_sb[:, k0 - KH, sl])
                                    r1 = (xt_sb[:, k1, sl] if k1 < KH
                                          else ht_sb[:, k1 - KH, sl])
                                    nc.tensor.matmul(
                                        pg[(b2, 0)], w8s[:, s, :, 0:128], rhs8,
                                        start=(s == 0), stop=(s == KG // 2 - 1),
                                        perf_mode=DR)
                                    nc.tensor.matmul(
                                        pg[(b2, 1)], wgs[:, k0, 128:256], r0,
                                        start=(s == 0), stop=False)
                                    nc.tensor.matmul(
                                        pg[(b2, 3)], wgs[:, k0, 384:512], r0,
                                        start=(s == 0), stop=False)
                                    nc.tensor.matmul(
                                        pg[(b2, 2)], w8s[:, s, :, 128:256], rhs8,
                                        start=(s == 0), stop=(s == KG // 2 - 1),
                                        perf_mode=DR)
                                    nc.tensor.matmul(
                                        pg[(b2, 1)], wgs[:, k1, 128:256], r1,
                                        start=False, stop=(s == KG // 2 - 1))
                                    nc.tensor.matmul(
                                        pg[(b2, 3)], wgs[:, k1, 384:512], r1,
                                        start=False, stop=(s == KG // 2 - 1))
                        else:
                            for kt in range(KG):
                                for b2 in range(2):
                                    b4 = bp * 2 + b2
                                    sl = slice(b4 * BC, (b4 + 1) * BC)
                                    rhs = (xt_sb[:, kt, sl] if kt < KH
                                           else ht_sb[:, kt - KH, sl])
                                    for g in range(4):
                                        nc.tensor.matmul(
                                            pg[(b2, g)],
                                            wgs[:, kt, g * 128:(g + 1) * 128],
                                            rhs,
                                            start=(kt == 0), stop=(kt == KG - 1))
                        if noact:
                            continue
                        if noact:
                            continue
                        for b2 in range(2):
                            sl2 = slice(b2 * BC, (b2 + 1) * BC)
                            for g in (0, 1, 3, 2):
                                scale = (1.0 / WS if fp8io and g in (0, 2)
                                         else 1.0)
                                nc.scalar.activation(
                                    gtiles[g][:, sl2], pg[(b2, g)], gfunc[g],
                                    bias=bg_sb[:, g * NJ + j:g * NJ + j + 1],
                                    scale=scale)
                    if noact:
                        continue
                    # elementwise after both eviction groups so the ACT FIFO
                    # is never parked behind a cross-engine round trip:
                    # c = f*cp + i*g; hpre = o*tanh(c)
                    for slh, it_, ft_, ot_, gt_ in halves:
                        cph = cp[:, slh]
                        nc.vector.tensor_mul(gt_, gt_, it_)      # i*g
                        nc.gpsimd.tensor_mul(cph, cph, ft_)      # f*cp
                        nc.vector.tensor_add(cph, cph, gt_)      # c
                        nc.sync.dma_start(co[j][:, slh], cph)
                    for slh, it_, ft_, ot_, gt_ in halves:
                        cph = cp[:, slh]
                        nc.scalar.activation(it_, cph, AF.Tanh)  # tanh(c)
                        nc.gpsimd.tensor_mul(hpre[:, j, slh], ot_, it_)

                if mmonly:
                    continue
                # ================= Phase B: softmax over batch ==============
                lps = [psp.tile([128, BC], F32, tag="ps", name=f"lps{b4}")
                       for b4 in range(NB)]
                for b4 in range(NB):
                    sl = slice(b4 * BC, (b4 + 1) * BC)
                    for j in range(NJ):
                        nc.tensor.matmul(
                            lps[b4][:1, :], wa_sb[:, j:j + 1], hpre[:, j, sl],
                            start=(j == 0), stop=(j == NJ - 1),
                        )
                exp_l = small.tile([1, BL], BF16, tag="exp_l")
                s_parts = small.tile([1, NB], F32, tag="s_parts")
                for b4 in range(NB):
                    sl = slice(b4 * BC, (b4 + 1) * BC)
                    nc.scalar.activation(exp_l[:, sl], lps[b4][:1, :], AF.Exp,
                                         bias=ba_sb,
                                         accum_out=s_parts[:, b4:b4 + 1])
                nc.gpsimd.partition_broadcast(expl_bc, exp_l)
                s_loc = small.tile([1, 1], F32, tag="s_loc")
                nc.vector.reduce_sum(s_loc, s_parts, axis=mybir.AxisListType.X)
                s_glob = small.tile([1, 1], F32, tag="s_glob")
                cc_out = None
                if single or nocc:
                    nc.vector.tensor_copy(s_glob, s_loc)
                else:
                    cc_in = dramp.tile([1, 1], F32, tag="cc_in")
                    cc_out = dramp.tile([1, 1], F32, tag="cc_out")
                    nc.sync.dma_start(cc_in, s_loc)
                    nc.gpsimd.collective_compute(
                        "AllReduce", ALU.add,
                        replica_groups=[list(range(NCORES))],
                        ins=[cc_in.opt()],
                        outs=[cc_out.opt()],
                    )

                # ====== Phase C1: highway GEMMs + pre-S merge (no 1/S) ======
                c_tts = []
                for j in range(NJ):
                    whs = whp.tile([128, KH, 256], BF16, tag="whs")
                    nc.sync.dma_start(whs, wh[j].rearrange("k p n -> p k n"))
                    t_t = ttp.tile([128, BL], BF16, tag="tt")
                    s_t = stp.tile([128, BL], BF16, tag="st")
                    pt = [psp.tile([128, BC], F32, tag="ps",
                                   name=f"pt{j}_{b4}") for b4 in range(NB)]
                    pss = [psp.tile([128, BC], F32, tag="ps",
                                    name=f"pss{j}_{b4}") for b4 in range(NB)]
                    for kt in range(KH):
                        for b4 in range(NB):
                            sl = slice(b4 * BC, (b4 + 1) * BC)
                            nc.tensor.matmul(pt[b4], whs[:, kt, :128],
                                             xt_sb[:, kt, sl],
                                             start=(kt == 0), stop=(kt == KH - 1))
                            nc.tensor.matmul(pss[b4], whs[:, kt, 128:],
                                             xt_sb[:, kt, sl],
                                             start=(kt == 0), stop=(kt == KH - 1))
                    for b4 in range(NB):
                        sl = slice(b4 * BC, (b4 + 1) * BC)
                        nc.scalar.activation(t_t[:, sl], pt[b4], AF.Identity,
                                             bias=bh_sb[:, j:j + 1])
                        nc.scalar.activation(s_t[:, sl], pss[b4], AF.Sigmoid,
                                             bias=bh_sb[:, NJ + j:NJ + j + 1])
                    # pre-S merge pieces (no 1/S dependency)
                    nc.gpsimd.tensor_mul(hpre[:, j, :], hpre[:, j, :], expl_bc)
                    nc.vector.tensor_mul(t_t, t_t, s_t)              # s*t
                    nc.vector.tensor_scalar(s_t, s_t, -1.0, 1.0,
                                            ALU.mult, ALU.add)       # 1-s
                    nc.gpsimd.tensor_mul(hpre[:, j, :], hpre[:, j, :], s_t)
                    c_tts.append(t_t)

                # ====== 1/S readback — placed here so no engine FIFO is
                # parked on the collective while C1 work remains ======
                if cc_out is not None:
                    nc.sync.dma_start(s_glob, cc_out)
                r_s = small.tile([1, 1], F32, tag="r_s")
                nc.vector.reciprocal(r_s, s_glob)
                rs_bc = small.tile([128, 1], F32, tag="rs_bc")
                nc.gpsimd.partition_broadcast(rs_bc, r_s)

                # ====== Phase C2: h = s*t + (1-s)*hpre*exp(l) * (1/S) ======
                for j in range(NJ):
                    t_t = c_tts[j]
                    nc.vector.tensor_scalar_mul(hpre[:, j, :], hpre[:, j, :],
                                                rs_bc)
                    if j % 2 == 0:
                        nc.vector.tensor_add(t_t, t_t, hpre[:, j, :])
                    else:
                        nc.gpsimd.tensor_add(t_t, t_t, hpre[:, j, :])
                    nc.sync.dma_start(ho[j][:, :BL // 2], t_t[:, :BL // 2])
                    nc.sync.dma_start(ho[j][:, BL // 2:], t_t[:, BL // 2:])
    nc.compile()
    return nc


def _prep_inputs(x, h_prev, c_prev, W_i, b_i, W_f, b_f, W_o, b_o, W_g, b_g,
                 W_a, b_a, W_ht, b_ht, W_hg, b_hg):
    bf16 = ml_dtypes.bfloat16
    fp8 = ml_dtypes.float8_e4m3
    f32 = np.float32

    W_gates = np.concatenate([W_i, W_f, W_o, W_g], axis=0)          # [4H, IN+H]
    # wg[j, kt] = [128 k, 512] with the 4 gate blocks {j, NJ+j, 2NJ+j, 3NJ+j}
    wg = (W_gates.T.astype(f32)
          .reshape(KG, 128, 4, NJ, 128)     # [kt, k, g, j, m]
          .transpose(3, 0, 1, 2, 4)         # [j, kt, k, g, m]
          .reshape(NJ, KG, 128, 512)
          .astype(bf16))
    # fp8 DoubleRow slab for i,o: w8[j, s, k, pair, {i,o}*128]
    W_io = np.stack([np.asarray(W_i, f32), np.asarray(W_o, f32)])   # [2, H, K]
    w8 = (W_io.transpose(2, 0, 1)          # [K, 2, H]
          .reshape(KG // 2, 2, 128, 2, NJ, 128)   # [s, pair, k, g8, j, m]
          .transpose(4, 0, 2, 1, 3, 5)             # [j, s, k, pair, g8, m]
          .reshape(NJ, KG // 2, 128, 2, 256))
    w8 = (w8 * WS).astype(fp8)
    W_h = np.concatenate([W_ht, W_hg], axis=0)                      # [2H, IN]
    wh = (W_h.T.astype(f32)
          .reshape(KH, 128, 2, NJ, 128)
          .transpose(3, 0, 1, 2, 4)
          .reshape(NJ, KH, 128, 256)
          .astype(bf16))
    wa = np.ascontiguousarray(
        np.asarray(W_a, f32).reshape(NJ, 128).T).astype(bf16)       # [128, NJ]
    bgp = (np.concatenate([b_i, b_f, b_o, b_g])
           .reshape(4, NJ, 128).transpose(2, 0, 1).reshape(128, 4 * NJ))
    bhp = (np.concatenate([b_ht, b_hg])
           .reshape(2, NJ, 128).transpose(2, 0, 1).reshape(128, 2 * NJ))
    bap = np.asarray(b_a, f32).reshape(1, 1)

    x = np.asarray(x, f32)
    h_prev = np.asarray(h_prev, f32)
    c_prev = np.asarray(c_prev, f32)

    in_maps = []
    for c in range(NCORES):
        rows = slice(c * BL, (c + 1) * BL)
        xtc = np.ascontiguousarray(x[rows].T).reshape(KH, 128, BL)
        htc = np.ascontiguousarray(h_prev[rows].T).reshape(NJ, 128, BL)
        ctc = np.ascontiguousarray(c_prev[rows].T).reshape(NJ, 128, BL)
        in_maps.append({
            "xt": xtc.astype(bf16), "ht": htc.astype(bf16),
            "ct": ctc.astype(bf16),
            "x8": xtc.astype(fp8), "h8": htc.astype(fp8),
            "wg": wg, "wh": wh, "wa": wa, "w8": w8,
            "bg": np.ascontiguousarray(bgp, f32),
            "bh": np.ascontiguousarray(bhp, f32),
            "ba": bap,
        })
    return in_maps


def kernel(**inputs):
    if "nc" not in _cached:
        _cached["nc"] = build_program()
    nc = _cached["nc"]
    in_maps = _prep_inputs(**inputs)
    res = run_bass_kernel_spmd(nc, in_maps, core_ids=list(range(NCORES)))
    h = np.empty((B, H), np.float32)
    c = np.empty((B, H), np.float32)
    for ci, out in enumerate(res.results):
        rows = slice(ci * BL, (ci + 1) * BL)
        h[rows] = out["ho"].reshape(H, BL).T.astype(np.float32)
        c[rows] = out["co"].reshape(H, BL).T.astype(np.float32)
    return (h, c)
